# revision 10
# baseline (speedup 1.0000x reference)
"""Multi-head causal attention with RoPE on 8 Trainium2 NeuronCores.

Sharding: core = batch(2) x head-group(4).  Each core computes the q/k/v
projections for its 4 heads (256 of 1024 channels), RoPE, causal attention,
and a partial o_proj against its 256 rows of Wo^T; the host sums the 4
partials per batch element.

Device layouts (per core):
  xT       [1024, 2048] f32r   x[b].T
  wqT/wkT/wvT [128, 8*256] f32r  K-block-major W.T slices (wq pre-scaled 1/8)
  woT      [128, 2*1024] bf16  c-block-major Wo[:, g].T
  cosT2/sinT2 [128, 2048] f32r  rope tables, stacked twice (head pair rows)
  rotT     [128, 128]  f32r    blockdiag(R,R).T, R = rotate_half matrix
  triu/ident [128, 128] bf16
  out      [2048, 1024] f32    partial (x @ Wo_g partial), host-summed

Attention per head h (Dh=64): scoresT tiles [s_k 128, s_q 1024] = kT.T@qT
(fp32r), exp -> bf16 sbuf, attn@v natural via ones-column in v (softmax
denominator rides along as column 64 of the psum), per-partition normalize,
PE-transpose of attn_out, o_proj in bf16.
"""
import os
import sys

sys.path.insert(0, "/opt/trn_rl_repo")

import numpy as np
import ml_dtypes

import concourse.bacc as bacc
import concourse.mybir as mybir
from concourse import tile
from concourse.bass_utils import run_bass_kernel_spmd

F32 = mybir.dt.float32
F32R = mybir.dt.float32r
BF16 = mybir.dt.bfloat16

D_MODEL = 1024
N_HEADS = 16
HEAD_DIM = 64
SEQ = 2048
BATCH = 2
ROPE_THETA = 10000.0

NB = SEQ // 128          # 16 s-blocks of 128
NSUP = SEQ // 1024       # 2 s-supers of 1024
HPG = 4                  # heads per group (per core)
CPG = HPG * HEAD_DIM     # 256 channels per group

_CACHE = {}
LAST_RESULT = None       # test harness reads exec_time_ns from here


def _build_nc(causal: bool):
    nc = bacc.Bacc("TRN2", target_bir_lowering=False, debug=False, num_devices=8)

    xT_d = nc.declare_dram_parameter("xT", [D_MODEL, SEQ], F32R, isOutput=False)
    wq_d = nc.declare_dram_parameter("wqT", [128, 8 * CPG], F32R, isOutput=False)
    wk_d = nc.declare_dram_parameter("wkT", [128, 8 * CPG], F32R, isOutput=False)
    wv_d = nc.declare_dram_parameter("wvT", [128, 8 * CPG], F32R, isOutput=False)
    wo_d = nc.declare_dram_parameter("woT", [128, 2 * D_MODEL], BF16, isOutput=False)
    cos_d = nc.declare_dram_parameter("cosT2", [128, SEQ], F32R, isOutput=False)
    sin_d = nc.declare_dram_parameter("sinT2", [128, SEQ], F32R, isOutput=False)
    rot_d = nc.declare_dram_parameter("rotT", [128, 128], F32R, isOutput=False)
    tri_d = nc.declare_dram_parameter("triu", [128, 128], F32, isOutput=False)
    id_d = nc.declare_dram_parameter("ident", [128, 128], BF16, isOutput=False)
    out_d = nc.declare_dram_parameter("out", [SEQ, D_MODEL], F32, isOutput=True)

    xT_r = xT_d.rearrange("(kb p) s -> p kb s", p=128)

    with tile.TileContext(nc) as tc:
        with (
            tc.tile_pool(name="res", bufs=1) as res,
            tc.tile_pool(name="ps", bufs=2, space="PSUM") as ps,
        ):
            # ---- resident constants ----
            wq_sb = res.tile([128, 8 * CPG], F32R)
            wk_sb = res.tile([128, 8 * CPG], F32R)
            wv_sb = res.tile([128, 8 * CPG], F32R)
            wo_sb = res.tile([128, 2 * D_MODEL], BF16)
            cos_sb = res.tile([128, SEQ], F32R)
            sin_sb = res.tile([128, SEQ], F32R)
            rot_sb = res.tile([128, 128], F32R)
            tri_sb = res.tile([128, 128], F32)
            id_sb = res.tile([128, 128], BF16)
            nc.sync.dma_start(wq_sb[:], wq_d[:])
            nc.sync.dma_start(wk_sb[:], wk_d[:])
            nc.sync.dma_start(wv_sb[:], wv_d[:])
            nc.sync.dma_start(wo_sb[:], wo_d[:])
            nc.sync.dma_start(cos_sb[:], cos_d[:])
            nc.sync.dma_start(sin_sb[:], sin_d[:])
            nc.sync.dma_start(rot_sb[:], rot_d[:])
            nc.sync.dma_start(tri_sb[:], tri_d[:])
            nc.sync.dma_start(id_sb[:], id_d[:])

            # ---- resident activations ----
            qf = res.tile([128, 2 * SEQ], F32R)          # [pair rows, pr*SEQ + s]
            kf = res.tile([128, 2 * SEQ], F32R)
            v_sb = res.tile([128, NB, HPG * 65], BF16)   # per s-block, head-slot 65 cols
            attn = res.tile([128, NB, CPG], BF16)        # attn out, natural [s, c]
            nc.vector.memset(v_sb[:, :, 64 : HPG * 65 : 65], 1.0)

            # prewarm the ACT exp table during the DMA/proj phase
            warm = res.tile([128, 1], F32)
            warm2 = res.tile([128, 1], BF16)
            nc.vector.memset(warm[:], 0.0)
            nc.scalar.activation(warm2[:], warm[:], mybir.ActivationFunctionType.Exp)

            # ================= projections + rope =================
            with tc.tile_pool(name="proj", bufs=2) as proj:
                for sup in range(NSUP):
                    s0 = sup * 1024
                    xp = []
                    for kb in range(8):
                        xt = proj.tile([128, 1024], F32R, name=f"xt{sup}_{kb}", tag="xt", bufs=9)
                        nc.sync.dma_start(xt[:], xT_r[:, kb, s0 : s0 + 1024])
                        xp.append(xt)
                    for tens, (w_sb, outf) in enumerate(((wq_sb, qf), (wk_sb, kf))):
                        for pr in range(2):
                            psq = ps.tile([128, 1024], F32, name="psq", tag="big")
                            for kb in range(8):
                                lhs = w_sb[:, kb * CPG + pr * 128 : kb * CPG + (pr + 1) * 128]
                                for nh in range(2):
                                    nc.tensor.matmul(
                                        psq[:, nh * 512 : (nh + 1) * 512],
                                        lhs,
                                        xp[kb][:, nh * 512 : (nh + 1) * 512],
                                        start=(kb == 0),
                                        stop=(kb == 7),
                                    )
                            qraw = proj.tile([128, 1024], F32R, name="qraw", tag="qraw")
                            nc.vector.tensor_copy(qraw[:], psq[:])
                            psr = ps.tile([128, 1024], F32, name="psr", tag="big")
                            for nh in range(2):
                                nc.tensor.matmul(
                                    psr[:, nh * 512 : (nh + 1) * 512],
                                    rot_sb[:],
                                    qraw[:, nh * 512 : (nh + 1) * 512],
                                    start=True,
                                    stop=True,
                                )
                            t1 = proj.tile([128, 1024], F32R, name="t1", tag="t1")
                            nc.vector.tensor_mul(t1[:], qraw[:], cos_sb[:, s0 : s0 + 1024])
                            t2 = proj.tile([128, 1024], F32R, name="t2", tag="t2")
                            nc.vector.tensor_mul(t2[:], psr[:], sin_sb[:, s0 : s0 + 1024])
                            dst = outf[:, pr * SEQ + s0 : pr * SEQ + s0 + 1024]
                            nc.vector.tensor_add(dst, t1[:], t2[:])
                    for sbi in range(8):
                        blk = sup * 8 + sbi
                        psv = ps.tile([128, CPG], F32, name="psv", tag="small")
                        for kb in range(8):
                            nc.tensor.matmul(
                                psv[:],
                                xp[kb][:, sbi * 128 : (sbi + 1) * 128],
                                wv_sb[:, kb * CPG : (kb + 1) * CPG],
                                start=(kb == 0),
                                stop=(kb == 7),
                            )
                        nc.vector.tensor_copy(
                            v_sb[:, blk, :].rearrange("p (h c) -> p h c", h=HPG)[:, :, 0:64],
                            psv[:].rearrange("p (h c) -> p h c", h=HPG),
                        )

            # ================= attention =================
            # scoresT tiles [s_k 128, s_q 1024] -> (mask diag on psum, exp->bf16,
            # zero left) -> attn@v transposed: psum[65, 1024] = sum_i v_ext_i.T @ et_i
            # (row 64 = softmax denominators) -> copy to sbuf (ACT) -> per-128-block
            # PE transpose to natural [s_q, 65] -> reciprocal + scalar-mul normalize.
            with tc.tile_pool(name="att", bufs=1) as att:
                for h in range(HPG):
                    pr, off = h // 2, (h % 2) * 64
                    qT_h = qf[off : off + 64, pr * SEQ : (pr + 1) * SEQ]
                    kT_h = kf[off : off + 64, pr * SEQ : (pr + 1) * SEQ]
                    for J in range(2):
                        n_i = 8 * J + 8 if causal else NB
                        exps = []
                        for i in range(n_i):
                            t = i - 8 * J
                            col0 = max(t, 0) * 128 if causal else 0
                            pss = ps.tile([128, 1024], F32, name="pss", tag="big")
                            for nh in range(2):
                                if (nh + 1) * 512 <= col0:
                                    continue
                                nc.tensor.matmul(
                                    pss[:, nh * 512 : (nh + 1) * 512],
                                    kT_h[:, i * 128 : (i + 1) * 128],
                                    qT_h[:, J * 1024 + nh * 512 : J * 1024 + (nh + 1) * 512],
                                    start=True,
                                    stop=True,
                                )
                            if causal and t >= 0:
                                # additive -1e4 on the masked (upper-left) part of the
                                # diagonal 128-block, before exp
                                nc.vector.tensor_add(
                                    pss[:, col0 : col0 + 128],
                                    pss[:, col0 : col0 + 128],
                                    tri_sb[:],
                                )
                            et = att.tile([128, 1024], BF16, name=f"et{h}_{J}_{i}", tag="et", bufs=18)
                            nc.scalar.activation(
                                et[:, col0:1024],
                                pss[:, col0:1024],
                                mybir.ActivationFunctionType.Exp,
                            )
                            if col0 > 0:
                                nc.gpsimd.memset(et[:, 0:col0], 0.0)
                            exps.append(et)
                        psuo = ps.tile([65, 1024], F32, name="psuo", tag="uo", bufs=1)
                        for i in range(n_i):
                            for nh in range(2):
                                nc.tensor.matmul(
                                    psuo[:, nh * 512 : (nh + 1) * 512],
                                    v_sb[:, i, h * 65 : h * 65 + 65],
                                    exps[i][:, nh * 512 : (nh + 1) * 512],
                                    start=(i == 0),
                                    stop=(i == n_i - 1),
                                )
                        uoT = att.tile([65, 1024], BF16, name="uoT", tag="uoT", bufs=2)
                        nc.scalar.copy(uoT[:], psuo[:])
                        for t in range(8):
                            j = 8 * J + t
                            pnat = ps.tile([128, 65], BF16, name="pnat", tag="small")
                            nc.tensor.transpose(
                                pnat[:], uoT[:, t * 128 : (t + 1) * 128], id_sb[0:65, 0:65]
                            )
                            rec = att.tile([128, 1], F32, name="rec", tag="rec", bufs=3)
                            nc.vector.reciprocal(rec[:], pnat[:, 64:65])
                            nc.vector.tensor_scalar_mul(
                                attn[:, j, h * 64 : (h + 1) * 64], pnat[:, 0:64], rec[:]
                            )

            # ================= o_proj =================
            with tc.tile_pool(name="oo", bufs=2) as oo:
                for j in range(NB):
                    ats = []
                    for cb in range(2):
                        ptt = ps.tile([128, 128], BF16, name="ptt", tag="small")
                        nc.tensor.transpose(ptt[:], attn[:, j, cb * 128 : (cb + 1) * 128], id_sb[:])
                        at_sb = oo.tile([128, 128], BF16, name="at", tag="at")
                        nc.vector.tensor_copy(at_sb[:], ptt[:])
                        ats.append(at_sb)
                    pso = ps.tile([128, 1024], F32, name="pso", tag="big")
                    for ds in range(2):
                        for cb in range(2):
                            nc.tensor.matmul(
                                pso[:, ds * 512 : (ds + 1) * 512],
                                ats[cb][:],
                                wo_sb[:, cb * D_MODEL + ds * 512 : cb * D_MODEL + (ds + 1) * 512],
                                start=(cb == 0),
                                stop=(cb == 1),
                            )
                    osb = oo.tile([128, 1024], F32, name="osb", tag="osb", bufs=3)
                    nc.scalar.copy(osb[:], pso[:])
                    nc.sync.dma_start(out_d[j * 128 : (j + 1) * 128, :], osb[:])

    nc.compile()
    return nc


def _host_tables():
    inv_freq = 1.0 / (ROPE_THETA ** (np.arange(0, HEAD_DIM, 2, dtype=np.float64) / HEAD_DIM))
    ang = np.arange(SEQ, dtype=np.float64)[:, None] * inv_freq[None, :]  # [S, 32]
    cos_h = np.cos(ang)
    sin_h = np.sin(ang)
    cos_full = np.concatenate([cos_h, cos_h], axis=1).astype(np.float32)  # [S, 64]
    sin_full = np.concatenate([sin_h, sin_h], axis=1).astype(np.float32)
    cosT2 = np.ascontiguousarray(np.vstack([cos_full.T, cos_full.T]))  # [128, S]
    sinT2 = np.ascontiguousarray(np.vstack([sin_full.T, sin_full.T]))
    # rotate_half matrix R [64,64]: (Rq)[j] = -q[j+32] (j<32), q[j-32] (j>=32)
    R = np.zeros((64, 64), np.float32)
    for jj in range(32):
        R[jj, jj + 32] = -1.0
        R[jj + 32, jj] = 1.0
    Rp = np.zeros((128, 128), np.float32)
    Rp[0:64, 0:64] = R
    Rp[64:128, 64:128] = R
    rotT = np.ascontiguousarray(Rp.T)
    return cosT2, sinT2, rotT


def _kb_major(wT):
    # [1024, C] -> [128, 8*C] with K-block-major columns
    C = wT.shape[1]
    return np.ascontiguousarray(wT.reshape(8, 128, C).transpose(1, 0, 2).reshape(128, 8 * C))


def _np_reference(x, mask, Wq, Wk, Wv, Wo):
    B, S, D = x.shape
    cosT2, sinT2, _ = _host_tables()
    cos = cosT2[:64].T[None, :, None, :]  # [1,S,1,64]
    sin = sinT2[:64].T[None, :, None, :]
    q = (x @ Wq.T).reshape(B, S, N_HEADS, HEAD_DIM)
    k = (x @ Wk.T).reshape(B, S, N_HEADS, HEAD_DIM)
    v = (x @ Wv.T).reshape(B, S, N_HEADS, HEAD_DIM)

    def rot(t):
        return np.concatenate([-t[..., 32:], t[..., :32]], axis=-1)

    q = q * cos + rot(q) * sin
    k = k * cos + rot(k) * sin
    sc = np.einsum("bqhd,bkhd->bhqk", q, k) / np.sqrt(HEAD_DIM)
    sc = np.where(mask[None, None], -np.inf, sc)
    sc = sc - sc.max(-1, keepdims=True)
    e = np.exp(sc)
    a = e / e.sum(-1, keepdims=True)
    o = np.einsum("bhqk,bkhd->bqhd", a, v).reshape(B, S, D)
    return (o @ Wo.T).astype(np.float32)


def kernel(x, mask, Wq, Wk, Wv, Wo):
    global LAST_RESULT
    x = np.asarray(x, np.float32)
    mask = np.asarray(mask, bool)
    Wq = np.asarray(Wq, np.float32)
    Wk = np.asarray(Wk, np.float32)
    Wv = np.asarray(Wv, np.float32)
    Wo = np.asarray(Wo, np.float32)

    causal_mask = np.triu(np.ones((SEQ, SEQ), bool), 1)
    if np.array_equal(mask, causal_mask):
        causal = True
    elif not mask.any():
        causal = False
    else:
        return _np_reference(x, mask, Wq, Wk, Wv, Wo)

    if causal not in _CACHE:
        _CACHE[causal] = _build_nc(causal)
    nc = _CACHE[causal]

    cosT2, sinT2, rotT = _host_tables()
    # additive mask for the diagonal 128-block: 0 where q>=k (col>=row), -1e4 else
    triu = np.where(
        np.arange(128)[None, :] >= np.arange(128)[:, None], 0.0, -1.0e4
    ).astype(np.float32)
    ident = np.eye(128, dtype=np.float32).astype(ml_dtypes.bfloat16)

    in_maps = []
    for b in range(BATCH):
        xT = np.ascontiguousarray(x[b].T)
        for g in range(4):
            sl = slice(g * CPG, (g + 1) * CPG)
            in_maps.append(
                {
                    "xT": xT,
                    "wqT": _kb_major(np.ascontiguousarray((Wq[sl] / np.sqrt(HEAD_DIM)).T)),
                    "wkT": _kb_major(np.ascontiguousarray(Wk[sl].T)),
                    "wvT": _kb_major(np.ascontiguousarray(Wv[sl].T)),
                    "woT": np.ascontiguousarray(
                        Wo[:, sl].T.reshape(2, 128, D_MODEL).transpose(1, 0, 2).reshape(128, 2 * D_MODEL)
                    ).astype(ml_dtypes.bfloat16),
                    "cosT2": cosT2,
                    "sinT2": sinT2,
                    "rotT": rotT,
                    "triu": triu,
                    "ident": ident,
                }
            )

    trace = os.environ.get("KERNEL_TRACE", "0") == "1"
    res = run_bass_kernel_spmd(nc, in_maps, list(range(8)), trace=trace)
    LAST_RESULT = res

    out = np.zeros((BATCH, SEQ, D_MODEL), np.float32)
    for b in range(BATCH):
        for g in range(4):
            out[b] += res.results[b * 4 + g]["out"]
    return out


# revision 14
# speedup vs baseline: 1.3543x; 1.3543x over previous
"""Multi-head causal attention with RoPE on 8 Trainium2 NeuronCores.

Sharding: core = batch(2) x head-group(4).  Each core computes the q/k/v
projections for its 4 heads (256 of 1024 channels), RoPE, causal attention,
and a partial o_proj against its 256 rows of Wo^T; the host sums the 4
partials per batch element.

Device layouts (per core):
  xT       [1024, 2048] f32r   x[b].T
  wqT/wkT/wvT [128, 8*256] f32r  K-block-major W.T slices (wq pre-scaled 1/8)
  woT      [128, 2*1024] bf16  c-block-major Wo[:, g].T
  cosT2/sinT2 [128, 2048] f32r  rope tables, stacked twice (head pair rows)
  rotT     [128, 128]  f32r    blockdiag(R,R).T, R = rotate_half matrix
  triu/ident [128, 128] bf16
  out      [2048, 1024] f32    partial (x @ Wo_g partial), host-summed

Attention per head h (Dh=64): scoresT tiles [s_k 128, s_q 1024] = kT.T@qT
(fp32r), exp -> bf16 sbuf, attn@v natural via ones-column in v (softmax
denominator rides along as column 64 of the psum), per-partition normalize,
PE-transpose of attn_out, o_proj in bf16.
"""
import os
import sys

sys.path.insert(0, "/opt/trn_rl_repo")

import numpy as np
import ml_dtypes

import concourse.bacc as bacc
import concourse.mybir as mybir
from concourse import tile
from concourse.bass_utils import run_bass_kernel_spmd

F32 = mybir.dt.float32
F32R = mybir.dt.float32r
BF16 = mybir.dt.bfloat16

D_MODEL = 1024
N_HEADS = 16
HEAD_DIM = 64
SEQ = 2048
BATCH = 2
ROPE_THETA = 10000.0

NB = SEQ // 128          # 16 s-blocks of 128
NSUP = SEQ // 1024       # 2 s-supers of 1024
HPG = 4                  # heads per group (per core)
CPG = HPG * HEAD_DIM     # 256 channels per group

_CACHE = {}
LAST_RESULT = None       # test harness reads exec_time_ns from here


def _build_nc(causal: bool):
    nc = bacc.Bacc("TRN2", target_bir_lowering=False, debug=False, num_devices=8)

    xT_d = nc.declare_dram_parameter("xT", [D_MODEL, SEQ], F32R, isOutput=False)
    wq_d = nc.declare_dram_parameter("wqT", [128, 8 * CPG], F32R, isOutput=False)
    wk_d = nc.declare_dram_parameter("wkT", [128, 8 * CPG], F32R, isOutput=False)
    wv_d = nc.declare_dram_parameter("wvT", [128, 8 * CPG], F32R, isOutput=False)
    wo_d = nc.declare_dram_parameter("woT", [128, 2 * D_MODEL], BF16, isOutput=False)
    cos_d = nc.declare_dram_parameter("cosT2", [128, SEQ], F32R, isOutput=False)
    sin_d = nc.declare_dram_parameter("sinT2", [128, SEQ], F32R, isOutput=False)
    rot_d = nc.declare_dram_parameter("rotT", [128, 128], F32R, isOutput=False)
    tri_d = nc.declare_dram_parameter("triu", [128, 128], BF16, isOutput=False)
    id_d = nc.declare_dram_parameter("ident", [128, 128], BF16, isOutput=False)
    out_d = nc.declare_dram_parameter("out", [SEQ, D_MODEL], F32, isOutput=True)

    xT_r = xT_d.rearrange("(kb p) s -> p kb s", p=128)

    with tile.TileContext(nc) as tc:
        with (
            tc.tile_pool(name="res", bufs=1) as res,
            tc.tile_pool(name="ps", bufs=2, space="PSUM") as ps,
        ):
            # ---- resident constants ----
            wq_sb = res.tile([128, 8 * CPG], F32R)
            wk_sb = res.tile([128, 8 * CPG], F32R)
            wv_sb = res.tile([128, 8 * CPG], F32R)
            wo_sb = res.tile([128, 2 * D_MODEL], BF16)
            cos_sb = res.tile([128, SEQ], F32R)
            sin_sb = res.tile([128, SEQ], F32R)
            rot_sb = res.tile([128, 128], F32R)
            tri_sb = res.tile([128, 128], BF16)
            id_sb = res.tile([128, 128], BF16)
            nc.sync.dma_start(wq_sb[:], wq_d[:])
            nc.sync.dma_start(wk_sb[:], wk_d[:])
            nc.sync.dma_start(wv_sb[:], wv_d[:])
            nc.sync.dma_start(wo_sb[:], wo_d[:])
            nc.sync.dma_start(cos_sb[:], cos_d[:])
            nc.sync.dma_start(sin_sb[:], sin_d[:])
            nc.sync.dma_start(rot_sb[:], rot_d[:])
            nc.sync.dma_start(tri_sb[:], tri_d[:])
            nc.sync.dma_start(id_sb[:], id_d[:])

            # ---- resident activations ----
            qf = res.tile([128, 2 * SEQ], F32R)          # [pair rows, pr*SEQ + s]
            kf = res.tile([128, 2 * SEQ], F32R)
            v_sb = res.tile([128, NB, HPG * 65], BF16)   # per s-block, head-slot 65 cols
            attn = res.tile([128, NB, CPG], BF16)        # attn out, natural [s, c]
            nc.vector.memset(v_sb[:, :, 64 : HPG * 65 : 65], 1.0)

            # prewarm the ACT exp table during the DMA/proj phase
            warm = res.tile([128, 1], F32)
            warm2 = res.tile([128, 1], BF16)
            nc.vector.memset(warm[:], 0.0)
            nc.scalar.activation(warm2[:], warm[:], mybir.ActivationFunctionType.Exp)

            # ================= projections + rope =================
            with tc.tile_pool(name="proj", bufs=2) as proj:
                for sup in range(NSUP):
                    s0 = sup * 1024
                    xp = []
                    for kb in range(8):
                        xt = proj.tile([128, 1024], F32R, name=f"xt{sup}_{kb}", tag="xt", bufs=9)
                        nc.sync.dma_start(xt[:], xT_r[:, kb, s0 : s0 + 1024])
                        xp.append(xt)
                    for tens, (w_sb, outf) in enumerate(((wq_sb, qf), (wk_sb, kf))):
                        for pr in range(2):
                            psq = ps.tile([128, 1024], F32, name="psq", tag="big", bufs=3)
                            for kb in range(8):
                                lhs = w_sb[:, kb * CPG + pr * 128 : kb * CPG + (pr + 1) * 128]
                                for nh in range(2):
                                    nc.tensor.matmul(
                                        psq[:, nh * 512 : (nh + 1) * 512],
                                        lhs,
                                        xp[kb][:, nh * 512 : (nh + 1) * 512],
                                        start=(kb == 0),
                                        stop=(kb == 7),
                                    )
                            qraw = proj.tile([128, 1024], F32R, name="qraw", tag="qraw")
                            nc.vector.tensor_copy(qraw[:], psq[:])
                            psr = ps.tile([128, 1024], F32, name="psr", tag="big", bufs=3)
                            for nh in range(2):
                                nc.tensor.matmul(
                                    psr[:, nh * 512 : (nh + 1) * 512],
                                    rot_sb[:],
                                    qraw[:, nh * 512 : (nh + 1) * 512],
                                    start=True,
                                    stop=True,
                                )
                            t1 = proj.tile([128, 1024], F32R, name="t1", tag="t1")
                            nc.vector.tensor_mul(t1[:], qraw[:], cos_sb[:, s0 : s0 + 1024])
                            t2 = proj.tile([128, 1024], F32R, name="t2", tag="t2")
                            nc.vector.tensor_mul(t2[:], psr[:], sin_sb[:, s0 : s0 + 1024])
                            dst = outf[:, pr * SEQ + s0 : pr * SEQ + s0 + 1024]
                            nc.vector.tensor_add(dst, t1[:], t2[:])
                    for sbi in range(8):
                        blk = sup * 8 + sbi
                        psv = ps.tile([128, CPG], F32, name="psv", tag="small")
                        for kb in range(8):
                            nc.tensor.matmul(
                                psv[:],
                                xp[kb][:, sbi * 128 : (sbi + 1) * 128],
                                wv_sb[:, kb * CPG : (kb + 1) * CPG],
                                start=(kb == 0),
                                stop=(kb == 7),
                            )
                        nc.vector.tensor_copy(
                            v_sb[:, blk, :].rearrange("p (h c) -> p h c", h=HPG)[:, :, 0:64],
                            psv[:].rearrange("p (h c) -> p h c", h=HPG),
                        )

            # ================= attention =================
            # scoresT tiles [s_k 128, s_q 1024] -> (mask diag on psum, exp->bf16,
            # zero left) -> attn@v transposed: psum[65, 1024] = sum_i v_ext_i.T @ et_i
            # (row 64 = softmax denominators) -> copy to sbuf (ACT) -> per-128-block
            # PE transpose to natural [s_q, 65] -> reciprocal + scalar-mul normalize.
            with tc.tile_pool(name="att", bufs=1) as att:
                for h in range(HPG):
                    pr, off = h // 2, (h % 2) * 64
                    qT_h = qf[off : off + 64, pr * SEQ : (pr + 1) * SEQ]
                    kT_h = kf[off : off + 64, pr * SEQ : (pr + 1) * SEQ]
                    for J in range(2):
                        n_i = 8 * J + 8 if causal else NB
                        exps = []
                        for i in range(n_i):
                            t = i - 8 * J
                            col0 = max(t, 0) * 128 if causal else 0
                            pss = ps.tile([128, 1024], F32, name="pss", tag="big", bufs=3)
                            diag_nh = col0 // 512
                            for nh in range(2):
                                if causal and (nh + 1) * 512 <= col0:
                                    continue
                                nc.tensor.matmul(
                                    pss[:, nh * 512 : (nh + 1) * 512],
                                    kT_h[:, i * 128 : (i + 1) * 128],
                                    qT_h[:, J * 1024 + nh * 512 : J * 1024 + (nh + 1) * 512],
                                    start=True,
                                    stop=not (causal and t >= 0 and nh == diag_nh),
                                )
                            if causal and t >= 0:
                                # additive -1e4 upper-left mask of the diagonal block,
                                # applied on the PE as an accumulating I.T @ triM matmul
                                nc.tensor.matmul(
                                    pss[:, col0 : col0 + 128],
                                    id_sb[:],
                                    tri_sb[:],
                                    start=False,
                                    stop=True,
                                    skip_group_check=True,
                                )
                            et = att.tile([128, 1024], BF16, name=f"et{h}_{J}_{i}", tag="et", bufs=26)
                            nc.scalar.activation(
                                et[:, col0:1024],
                                pss[:, col0:1024],
                                mybir.ActivationFunctionType.Exp,
                            )
                            # zero the garbage left of col0 that attn@v's nh-split
                            # would still read: [nh*512, col0) for the halves it uses
                            if causal and 0 < t <= 3:
                                nc.vector.memset(et[:, 0:col0], 0.0)
                            elif causal and t >= 5:
                                nc.vector.memset(et[:, 512:col0], 0.0)
                            exps.append(et)
                        psuo = ps.tile([65, 1024], F32, name="psuo", tag="big", bufs=3)
                        for nh in range(2):
                            n_nh = min(n_i, 8 * J + 4 * (nh + 1)) if causal else n_i
                            for i in range(n_nh):
                                nc.tensor.matmul(
                                    psuo[:, nh * 512 : (nh + 1) * 512],
                                    v_sb[:, i, h * 65 : h * 65 + 65],
                                    exps[i][:, nh * 512 : (nh + 1) * 512],
                                    start=(i == 0),
                                    stop=(i == n_nh - 1),
                                )
                        uoT = att.tile([65, 1024], BF16, name="uoT", tag="uoT", bufs=2)
                        nc.vector.tensor_copy(uoT[:], psuo[:])
                        for t in range(8):
                            j = 8 * J + t
                            pnat = ps.tile([128, 65], BF16, name="pnat", tag="small")
                            nc.tensor.transpose(
                                pnat[:], uoT[:, t * 128 : (t + 1) * 128], id_sb[0:65, 0:65]
                            )
                            rec = att.tile([128, 1], F32, name="rec", tag="rec", bufs=3)
                            nc.vector.reciprocal(rec[:], pnat[:, 64:65])
                            nc.vector.tensor_scalar_mul(
                                attn[:, j, h * 64 : (h + 1) * 64], pnat[:, 0:64], rec[:]
                            )

            # ================= o_proj =================
            with tc.tile_pool(name="oo", bufs=2) as oo:
                for j in range(NB):
                    ats = []
                    for cb in range(2):
                        ptt = ps.tile([128, 128], BF16, name="ptt", tag="small")
                        nc.tensor.transpose(ptt[:], attn[:, j, cb * 128 : (cb + 1) * 128], id_sb[:])
                        at_sb = oo.tile([128, 128], BF16, name="at", tag="at")
                        nc.vector.tensor_copy(at_sb[:], ptt[:])
                        ats.append(at_sb)
                    pso = ps.tile([128, 1024], F32, name="pso", tag="big", bufs=3)
                    for ds in range(2):
                        for cb in range(2):
                            nc.tensor.matmul(
                                pso[:, ds * 512 : (ds + 1) * 512],
                                ats[cb][:],
                                wo_sb[:, cb * D_MODEL + ds * 512 : cb * D_MODEL + (ds + 1) * 512],
                                start=(cb == 0),
                                stop=(cb == 1),
                            )
                    osb = oo.tile([128, 1024], F32, name="osb", tag="osb", bufs=3)
                    nc.scalar.copy(osb[:], pso[:])
                    nc.sync.dma_start(out_d[j * 128 : (j + 1) * 128, :], osb[:])

    nc.compile()
    return nc


def _host_tables():
    inv_freq = 1.0 / (ROPE_THETA ** (np.arange(0, HEAD_DIM, 2, dtype=np.float64) / HEAD_DIM))
    ang = np.arange(SEQ, dtype=np.float64)[:, None] * inv_freq[None, :]  # [S, 32]
    cos_h = np.cos(ang)
    sin_h = np.sin(ang)
    cos_full = np.concatenate([cos_h, cos_h], axis=1).astype(np.float32)  # [S, 64]
    sin_full = np.concatenate([sin_h, sin_h], axis=1).astype(np.float32)
    cosT2 = np.ascontiguousarray(np.vstack([cos_full.T, cos_full.T]))  # [128, S]
    sinT2 = np.ascontiguousarray(np.vstack([sin_full.T, sin_full.T]))
    # rotate_half matrix R [64,64]: (Rq)[j] = -q[j+32] (j<32), q[j-32] (j>=32)
    R = np.zeros((64, 64), np.float32)
    for jj in range(32):
        R[jj, jj + 32] = -1.0
        R[jj + 32, jj] = 1.0
    Rp = np.zeros((128, 128), np.float32)
    Rp[0:64, 0:64] = R
    Rp[64:128, 64:128] = R
    rotT = np.ascontiguousarray(Rp.T)
    return cosT2, sinT2, rotT


def _kb_major(wT):
    # [1024, C] -> [128, 8*C] with K-block-major columns
    C = wT.shape[1]
    return np.ascontiguousarray(wT.reshape(8, 128, C).transpose(1, 0, 2).reshape(128, 8 * C))


def _np_reference(x, mask, Wq, Wk, Wv, Wo):
    B, S, D = x.shape
    cosT2, sinT2, _ = _host_tables()
    cos = cosT2[:64].T[None, :, None, :]  # [1,S,1,64]
    sin = sinT2[:64].T[None, :, None, :]
    q = (x @ Wq.T).reshape(B, S, N_HEADS, HEAD_DIM)
    k = (x @ Wk.T).reshape(B, S, N_HEADS, HEAD_DIM)
    v = (x @ Wv.T).reshape(B, S, N_HEADS, HEAD_DIM)

    def rot(t):
        return np.concatenate([-t[..., 32:], t[..., :32]], axis=-1)

    q = q * cos + rot(q) * sin
    k = k * cos + rot(k) * sin
    sc = np.einsum("bqhd,bkhd->bhqk", q, k) / np.sqrt(HEAD_DIM)
    sc = np.where(mask[None, None], -np.inf, sc)
    sc = sc - sc.max(-1, keepdims=True)
    e = np.exp(sc)
    a = e / e.sum(-1, keepdims=True)
    o = np.einsum("bhqk,bkhd->bqhd", a, v).reshape(B, S, D)
    return (o @ Wo.T).astype(np.float32)


def kernel(x, mask, Wq, Wk, Wv, Wo):
    global LAST_RESULT
    x = np.asarray(x, np.float32)
    mask = np.asarray(mask, bool)
    Wq = np.asarray(Wq, np.float32)
    Wk = np.asarray(Wk, np.float32)
    Wv = np.asarray(Wv, np.float32)
    Wo = np.asarray(Wo, np.float32)

    causal_mask = np.triu(np.ones((SEQ, SEQ), bool), 1)
    if np.array_equal(mask, causal_mask):
        causal = True
    elif not mask.any():
        causal = False
    else:
        return _np_reference(x, mask, Wq, Wk, Wv, Wo)

    if causal not in _CACHE:
        _CACHE[causal] = _build_nc(causal)
    nc = _CACHE[causal]

    cosT2, sinT2, rotT = _host_tables()
    # additive mask for the diagonal 128-block: 0 where q>=k (col>=row), -1e4 else
    triu = np.where(
        np.arange(128)[None, :] >= np.arange(128)[:, None], 0.0, -1.0e4
    ).astype(np.float32).astype(ml_dtypes.bfloat16)
    ident = np.eye(128, dtype=np.float32).astype(ml_dtypes.bfloat16)

    in_maps = []
    for b in range(BATCH):
        xT = np.ascontiguousarray(x[b].T)
        for g in range(4):
            sl = slice(g * CPG, (g + 1) * CPG)
            in_maps.append(
                {
                    "xT": xT,
                    "wqT": _kb_major(np.ascontiguousarray((Wq[sl] / np.sqrt(HEAD_DIM)).T)),
                    "wkT": _kb_major(np.ascontiguousarray(Wk[sl].T)),
                    "wvT": _kb_major(np.ascontiguousarray(Wv[sl].T)),
                    "woT": np.ascontiguousarray(
                        Wo[:, sl].T.reshape(2, 128, D_MODEL).transpose(1, 0, 2).reshape(128, 2 * D_MODEL)
                    ).astype(ml_dtypes.bfloat16),
                    "cosT2": cosT2,
                    "sinT2": sinT2,
                    "rotT": rotT,
                    "triu": triu,
                    "ident": ident,
                }
            )

    trace = os.environ.get("KERNEL_TRACE", "0") == "1"
    res = run_bass_kernel_spmd(nc, in_maps, list(range(8)), trace=trace)
    LAST_RESULT = res

    out = np.zeros((BATCH, SEQ, D_MODEL), np.float32)
    for b in range(BATCH):
        for g in range(4):
            out[b] += res.results[b * 4 + g]["out"]
    return out
